# revision 10
# baseline (speedup 1.0000x reference)
"""Trainium2 Bass kernel for nn_Attention (B=1, C=64, 12x12x12 spatial, 32 heads, head_dim=2).

Sharding: 32 heads over 8 cores (4 heads/core), tensor-parallel: per-core
partial output summed on host with bias/8 folded per core.

v5 vs v4 (127us): 2+2 exp split, V-prime off the prologue critical path,
interleaved q/k staging, proj 12-13 overlapped into qt3.
- S-matmuls run 8-way concurrent: (32h, 64*parity) for 4 heads x 2 x 64-key
  chunks = all 16 PE sub-arrays; each pair-iter's S wall is ~qn columns.
- U-matmuls contract 128 keys per pass (halves U column-streams), 4-way
  col-tiled at (0, 32h).
- Two heads share one [128, 1024] f32 score tile (h at col offset qn*(h%2))
  so each exp instruction runs at FD=2*qn (amortizes engine overhead).
- Hybrid exp: ScalarE ACT exp for tile AB (+ every 4th CD), DVE Schraudolph
  (tensor_scalar f32->i16; bits are the bf16 exp approx) for the rest.
- Query tiles of 512 (PSUM bank limit): tiles AB/CD are 2 banks each,
  pool of 3 + pu 1 bank = 7 of 8 banks.
"""

import numpy as np
import ml_dtypes

import concourse.bass as bass
import concourse.bacc as bacc
import concourse.mybir as mybir
from concourse import tile
from concourse.bass_utils import run_bass_kernel_spmd

C = 64
N = 1728
NCORES = 8
HLOC = 4
SCALE = float(2.0 ** -0.5)

QTS = [(0, 512), (512, 512), (1024, 512), (1536, 192)]
PAIRS = [(128 * p, 128) for p in range(13)] + [(1664, 64)]
NPR = len(PAIRS)
QKCH = [(0, 512), (512, 512), (1024, 512), (1536, 192)]

LOG2E = 1.4426950408889634
A16 = SCALE * 128.0 * LOG2E
B16 = 127.0 * 128.0

F32 = mybir.dt.float32
BF16 = mybir.dt.bfloat16
I16 = mybir.dt.int16
EXP = mybir.ActivationFunctionType.Exp
MUL = mybir.AluOpType.mult
ADD = mybir.AluOpType.add


def build_nc(debug=False):
    nc = bacc.Bacc(None)

    x2 = nc.declare_dram_parameter("x2", [C, N], BF16, isOutput=False)
    w24 = nc.declare_dram_parameter("w24", [C, 3 * 2 * HLOC], BF16, isOutput=False)
    wp = nc.declare_dram_parameter("wp", [2 * HLOC + 1, C], F32, isOutput=False)
    y = nc.declare_dram_parameter("y", [N, C], F32, isOutput=True)
    if debug:
        d_qT = nc.declare_dram_parameter("d_qT", [128, N], BF16, isOutput=True)
        d_kT = nc.declare_dram_parameter("d_kT", [128, N], BF16, isOutput=True)
        d_vp = nc.declare_dram_parameter("d_vp", [128, NPR * 12], BF16, isOutput=True)
        d_ot = nc.declare_dram_parameter("d_ot", [9, N], F32, isOutput=True)
        d_u = nc.declare_dram_parameter("d_u", [128, 512], F32, isOutput=True)

    with tile.TileContext(nc) as tc:
        with (
            tc.tile_pool(name="const", bufs=1) as cpool,
            tc.tile_pool(name="epool", bufs=16) as epool,
            tc.tile_pool(name="dpool", bufs=2) as dpool,
            tc.tile_pool(name="ps", bufs=7, space=bass.MemorySpace.PSUM) as ps,
            tc.tile_pool(name="psu", bufs=1, space=bass.MemorySpace.PSUM) as psu,
        ):
            x_sb = cpool.tile([C, N], BF16, name="x_sb")
            w24_sb = cpool.tile([C, 24], BF16, name="w24_sb")
            wp_sb = cpool.tile([2 * HLOC + 1, C], F32, name="wp_sb")
            qT = cpool.tile([128, N], BF16, name="qT")
            kT = cpool.tile([128, N], BF16, name="kT")
            vp = cpool.tile([128, NPR * HLOC * 3], BF16, name="vp")
            ot = cpool.tile([2 * HLOC + 1, N], F32, name="ot")
            yb0 = cpool.tile([108, 512], F32, name="yb0")
            yb1 = cpool.tile([108, 512], F32, name="yb1")
            ybs = [yb0, yb1]

            nc.sync.dma_start(out=x_sb[:], in_=x2[:])
            nc.sync.dma_start(out=w24_sb[:], in_=w24[:])
            nc.sync.dma_start(out=wp_sb[:], in_=wp[:])

            vp_v = vp[:].rearrange("p (pr h c) -> p pr h c", h=HLOC, c=3)
            # ot rows 0..7 are overwritten by the divide unpack; row 8 stays 1.0
            nc.gpsimd.memset(ot[:, :], 1.0)
            nc.gpsimd.memset(vp_v[:, :, :, 2:3], 1.0)

            # ---- qkv projections, col-tiled 4 heads at once ----
            for ci, (co, cn) in enumerate(QKCH):
                for dst, wofs, eng in ((qT, 0, 0), (kT, 8, 1)):
                    pq = ps.tile([128, 512], F32, tag="s", name="pq")
                    for h in range(HLOC):
                        nc.tensor.matmul(
                            pq[32 * h : 32 * h + 2, :cn],
                            w24_sb[:, wofs + 2 * h : wofs + 2 * h + 2],
                            x_sb[:, co : co + cn],
                            start=True, stop=True,
                            tile_position=(0, 32 * h),
                        )
                    if (ci + eng) % 2 == 0:
                        nc.vector.tensor_copy(dst[:, co : co + cn], pq[:, :cn])
                    else:
                        nc.scalar.copy(dst[:, co : co + cn], pq[:, :cn])

            # ---- V': x key-chunk-pairs as weights -> [128, 8] per pair ----
            # (emitted from qt0's first hook so it doesn't delay the first S)
            def emit_vprime():
                psv = ps.tile([128, NPR * 8], F32, tag="s", name="psv")
                for pr, (ko, kn2) in enumerate(PAIRS):
                    nc.tensor.matmul(
                        psv[:kn2, 8 * pr : 8 * pr + 8],
                        x_sb[:, ko : ko + kn2],
                        w24_sb[:, 16:24],
                        start=True, stop=True,
                    )
                psv_v = psv[:].rearrange("p (pr h d) -> p pr h d", h=HLOC, d=2)
                nc.vector.tensor_copy(vp_v[:, :, :, 0:2], psv_v[:, :, :, :])

            # ---- main attention loops ----
            def emit_S(qo, qn, pr):
                # one MM per head: weight kT[2, kn2] covers the whole key
                # pair; 4 heads row-tiled at (32h, 0) run concurrently.
                ko, kn2 = PAIRS[pr]
                tiles = [ps.tile([128, 512], F32, tag="s", name=f"sc{h}")
                         for h in range(HLOC)]
                for h in range(HLOC):
                    nc.tensor.matmul(
                        tiles[h][:kn2, :qn],
                        kT[32 * h : 32 * h + 2, ko : ko + kn2],
                        qT[32 * h : 32 * h + 2, qo : qo + qn],
                        start=True, stop=True,
                        tile_position=(32 * h, 0),
                    )
                return tiles

            def emit_exp(qn, sc, n_scalar):
                es = [epool.tile([128, 512], BF16, tag="e", name=f"es{h}")
                      for h in range(HLOC)]
                for h in range(HLOC):
                    if h < n_scalar:
                        nc.scalar.activation(
                            es[h][:, :qn], sc[h][:, :qn], EXP, scale=SCALE)
                    else:
                        nc.vector.tensor_scalar(
                            es[h][:, :qn].bitcast(I16), sc[h][:, :qn],
                            A16, B16, MUL, ADD,
                        )
                return es

            def emit_U(qn, pr, es, pu):
                ko, kn2 = PAIRS[pr]
                for h in range(HLOC):
                    nc.tensor.matmul(
                        pu[32 * h : 32 * h + 3, 0:qn],
                        vp_v[:kn2, pr, h, :],
                        es[h][:kn2, :qn],
                        start=(pr == 0), stop=(pr == NPR - 1),
                        tile_position=(0, 32 * h),
                    )

            def emit_divide(qo, qn, pu):
                zz = dpool.tile([128, 512], F32, tag="zz", name="zz")
                zz_v = zz[:, :qn].rearrange("(h g) n -> h g n", g=32)
                nc.vector.reciprocal_approx_fast(zz[:, :qn], pu[:, :qn])
                nc.sync.dma_start(out=zz_v[:, 0, :], in_=zz_v[:, 2, :])
                nc.gpsimd.dma_start(out=zz_v[:, 1, :], in_=zz_v[:, 2, :])
                osp = dpool.tile([128, 512], F32, tag="osp", name="osp")
                nc.vector.tensor_mul(osp[:, :qn], pu[:, :qn], zz[:, :qn])
                ov = osp[:, :qn].rearrange("(h g) n -> h g n", g=32)
                ot_v = ot[0 : 2 * HLOC, qo : qo + qn].rearrange("(h d) n -> h d n", d=2)
                nc.sync.dma_start(out=ot_v[:, 0, :], in_=ov[:, 0, :])
                nc.gpsimd.dma_start(out=ot_v[:, 1, :], in_=ov[:, 1, :])
                return zz

            def emit_proj(tstart, nch, copy_eng=0):
                py = ps.tile([108, 512], F32, tag="s", name="py")
                for t8 in range(nch):
                    t = tstart + t8
                    nc.tensor.matmul(
                        py[:108, 64 * t8 : 64 * t8 + 64],
                        ot[:, 108 * t : 108 * t + 108],
                        wp_sb[:],
                        start=True, stop=True,
                    )
                yb = ybs[tstart // 8]
                o8 = (tstart % 8) * 64
                if copy_eng == 0:
                    nc.vector.tensor_copy(yb[:, o8 : o8 + 64 * nch], py[:108, : 64 * nch])
                else:
                    nc.scalar.copy(yb[:, o8 : o8 + 64 * nch], py[:108, : 64 * nch])

            def emit_proj_out(half):
                yb = ybs[half]
                yv = y[864 * half : 864 * (half + 1), :].rearrange(
                    "(t i) c -> i t c", i=108
                )
                nc.sync.dma_start(out=yv, in_=yb[:].rearrange("p (t c) -> p t c", c=64))

            def emit_y_part(t0, t1):
                yv = y[t0:t1, :].rearrange("(t i) c -> i t c", i=108)
                c0 = (t0 // 108 - 8) * 64
                c1 = (t1 // 108 - 8) * 64
                nc.sync.dma_start(
                    out=yv,
                    in_=ybs[1][:, c0:c1].rearrange("p (t c) -> p t c", c=64))

            def qt_loop(qo, qn, hooks):
                pu = psu.tile([128, 512], F32, tag="u", name="pu")
                pend = []
                for pr in range(NPR):
                    sc = emit_S(qo, qn, pr)
                    # lag U by TWO iters so its es inputs are never fresh
                    # (U-group leader otherwise stalls on the exp engines)
                    if len(pend) == 3:
                        p0 = pend.pop(0)
                        emit_U(qn, p0[0], p0[1], pu)
                    hk = hooks.get(pr)
                    if hk:
                        hk()
                    es = emit_exp(qn, sc, 2)
                    pend.append((pr, es))
                for p0 in pend:
                    emit_U(qn, p0[0], p0[1], pu)
                return pu

            for qi, (qo, qn) in enumerate(QTS):
                hooks = {}
                if qi == 0:
                    hooks = {0: emit_vprime}
                elif qi == 1:
                    hooks = {4: lambda: emit_proj(0, 2, 0),
                             7: lambda: emit_proj(2, 2, 1)}
                elif qi == 2:
                    hooks = {4: lambda: emit_proj(4, 2, 0),
                             7: lambda: emit_proj(6, 2, 1),
                             10: lambda: emit_proj_out(0)}
                elif qi == 3:
                    hooks = {4: lambda: emit_proj(8, 2, 0),
                             7: lambda: emit_proj(10, 2, 1),
                             10: lambda: emit_proj(12, 2, 0),
                             12: lambda: emit_y_part(864, 1512)}
                pu = qt_loop(qo, qn, hooks)
                z = emit_divide(qo, qn, pu)
                if qi == 0 and debug:
                    # dump immediately: the dpool buffer is recycled by later qts
                    nc.sync.dma_start(out=d_u[:], in_=z[:])
            emit_proj(14, 2, 1)
            emit_y_part(1512, 1728)

            if debug:
                nc.sync.dma_start(out=d_qT[:], in_=qT[:])
                nc.sync.dma_start(out=d_kT[:], in_=kT[:])
                nc.sync.dma_start(out=d_vp[:], in_=vp[:])
                nc.sync.dma_start(out=d_ot[:], in_=ot[:])

    return nc


_NC = None


def _get_nc():
    global _NC
    if _NC is None:
        _NC = build_nc()
        _NC.finalize()
    return _NC


def make_in_maps(x, w_qkv, w_proj, b_proj):
    x2 = np.ascontiguousarray(x.reshape(C, N)).astype(ml_dtypes.bfloat16)
    in_maps = []
    for c in range(NCORES):
        sl = slice(8 * c, 8 * c + 8)
        w24 = np.concatenate(
            [
                w_qkv[sl, :].T,
                w_qkv[64 + 8 * c : 64 + 8 * c + 8, :].T,
                w_qkv[128 + 8 * c : 128 + 8 * c + 8, :].T,
            ],
            axis=1,
        ).astype(ml_dtypes.bfloat16)
        wp = np.concatenate(
            [w_proj[:, sl].T, (b_proj / NCORES)[None, :]], axis=0
        ).astype(np.float32)
        in_maps.append(
            {
                "x2": x2,
                "w24": np.ascontiguousarray(w24),
                "wp": np.ascontiguousarray(wp),
            }
        )
    return in_maps


def run(x, w_qkv, w_proj, b_proj, trace=False, **kw):
    nc = _get_nc()
    in_maps = make_in_maps(x, w_qkv, w_proj, b_proj)
    res = run_bass_kernel_spmd(
        nc, in_maps, core_ids=list(range(NCORES)), trace=trace, **kw
    )
    y = np.zeros((N, C), np.float32)
    for r in res.results:
        y += r["y"]
    return y.reshape(1, 12, 12, 12, C), res


def kernel(x, w_qkv, w_proj, b_proj):
    out, _ = run(
        np.asarray(x), np.asarray(w_qkv), np.asarray(w_proj), np.asarray(b_proj)
    )
    return out
